# revision 8
# baseline (speedup 1.0000x reference)
"""Trainium2 Bass kernel for nn_AttentionMLP (B=4, S=4096, two attention+MLP stages).

Sharding: 8 cores = 4 batches x 2 sequence-halves. Each core computes its
2048 query rows end-to-end; a pairwise AllGather exchanges the stage-1
output halves so stage 2 can attend over the full sequence.

Layout strategy (per core, all feature-major / transposed):
  xT [64, S]   -> qT/kT [64, *] projections on PE (fp32r)
  scoresT[j, si] blocks via PE (K=64), exp on ACT into SBUF (fp32r)
  attn@v + rowsum fused: lhsT = [v | ones] [128jb, 65], accumulate in PSUM
  normalize via reciprocal_approx_fast + gpsimd partition_broadcast + DVE mul
  MLP: W1T/W2T matmuls, ELU = max(x+b,0) + exp(min(x+b,0)) - 1 (the -1 is
  folded into the next layer's bias), biases via K=1 ones-matmul into PSUM.
"""

import numpy as np
from contextlib import ExitStack

import concourse.bass as bass
import concourse.tile as tile
from concourse import bacc, mybir
from concourse import bass_utils

F32 = mybir.dt.float32
F32R = mybir.dt.float32r
EXP = mybir.ActivationFunctionType.Exp
ADD = mybir.AluOpType.add
MIN = mybir.AluOpType.min
MAX = mybir.AluOpType.max

N_CORES = 8
B, S, D = 4, 4096, 64
R = S // 2            # own query rows per core
HD = 256
NCK = R // 512        # si-chunks per core (4 x 512)
NJB = S // 128        # key blocks (32 x 128)
# exp-group sizes per chunk: PSUM allows a [128, 1536] double-buffered
# scores tile (6 banks) + 2 banks for everything else.
GROUPS = [3] * 10 + [2]
assert sum(GROUPS) == NJB


def _stage(nc, tc, ctx, pools, W, xT, q_src, outT=None, out_dram=None):
    """One attention+MLP stage. xT: SBUF [64, S] f32r (full sequence,
    feature-major). q_src: SBUF [64, R] f32r (own rows). Writes either
    outT (SBUF [64, R] f32r, stage 1) or out_dram ([R, 64] f32, stage 2)."""
    consts, sb, ps = pools
    wq, wk, wv, w1t, w2t, b1c, bias_lhsT, bias_rhs, ones512, ones128 = W

    # --- projections ---
    qT = sb.tile([64, R], F32R, tag="qT")
    kT = sb.tile([64, S], F32R, tag="kT")
    for n in range(S // 512):
        pk = ps.tile([64, 512], F32, tag="scores")
        nc.tensor.matmul(pk[:], wk[:], xT[:, n * 512:(n + 1) * 512],
                         start=True, stop=True)
        nc.vector.tensor_copy(kT[:, n * 512:(n + 1) * 512], pk[:])
    for n in range(R // 512):
        pq = ps.tile([64, 512], F32, tag="scores")
        nc.tensor.matmul(pq[:], wq[:], q_src[:, n * 512:(n + 1) * 512],
                         start=True, stop=True)
        nc.vector.tensor_copy(qT[:, n * 512:(n + 1) * 512], pq[:])

    # v rows + ones column: v_aug[j, 0:64] = (x @ Wv)[j], v_aug[j, 64] = 1
    # (memset can't emit f32r; go through an f32 scratch + DVE copy)
    v_aug = sb.tile([128, NJB, 65], F32R, tag="v_aug")
    onescol = sb.tile([128, NJB], F32, tag="onescol")
    nc.vector.memset(onescol[:], 1.0)
    nc.vector.tensor_copy(v_aug[:, :, 64:65], onescol[:].unsqueeze(2))
    for g in range(NJB // 4):
        pv = ps.tile([128, 4, 64], F32, tag="scores")
        for i in range(4):
            jb = g * 4 + i
            nc.tensor.matmul(pv[:, i, :], xT[:, jb * 128:(jb + 1) * 128],
                             wv[:], start=True, stop=True)
        nc.vector.tensor_copy(v_aug[:, g * 4:(g + 1) * 4, 0:64], pv[:])

    # --- per si-chunk: scores -> exp -> attn@v -> normalize -> MLP ---
    for n in range(NCK):
        qs = qT[:, n * 512:(n + 1) * 512]
        av = ps.tile([65, 512], F32, tag="avs")
        jb = 0
        for gsz in GROUPS:
            st = ps.tile([128, gsz * 512], F32, tag="scores")
            for i in range(gsz):
                nc.tensor.matmul(st[:, i * 512:(i + 1) * 512],
                                 kT[:, (jb + i) * 128:(jb + i + 1) * 128],
                                 qs, start=True, stop=True)
            ex = sb.tile([128, gsz * 512], F32R, tag="exp", bufs=3)
            nc.scalar.activation(ex[:], st[:], EXP)
            for i in range(gsz):
                nc.tensor.matmul(av[:], v_aug[:, jb + i, :],
                                 ex[:, i * 512:(i + 1) * 512],
                                 start=(jb + i == 0), stop=(jb + i == NJB - 1))
            jb += gsz

        # normalize: aT = av[0:64] / av[64]. partition_broadcast and the
        # custom recip op only work from partition 0, so move the row sums
        # there first (DVE copies handle cross-partition fine).
        rs = sb.tile([1, 512], F32, tag="rs", bufs=2)
        nc.vector.tensor_copy(rs[:], av[64:65, :])
        rr = sb.tile([1, 512], F32, tag="rr", bufs=2)
        nc.vector.reciprocal_approx_fast(rr[:], rs[:])
        rb = sb.tile([64, 512], F32, tag="rb", bufs=2)
        nc.gpsimd.partition_broadcast(rb[:], rr[:])
        aT = sb.tile([64, 512], F32R, tag="aT", bufs=2)
        nc.vector.tensor_mul(aT[:], av[0:64, :], rb[:])

        # MLP hidden: hT = elu(W1 @ aT + b1) + 1  (the -1 lives in b2_eff)
        u = sb.tile([128, 1024], F32, tag="u", bufs=2)
        r = sb.tile([128, 1024], F32, tag="r", bufs=2)
        for j in range(2):
            ph = ps.tile([128, 512], F32, tag="avs")
            nc.tensor.matmul(ph[:], w1t[:, j * 128:(j + 1) * 128], aT[:],
                             start=True, stop=True)
            nc.vector.tensor_scalar(u[:, j * 512:(j + 1) * 512], ph[:],
                                    b1c[:, j:j + 1], 0.0, op0=ADD, op1=MIN)
            nc.vector.tensor_scalar(r[:, j * 512:(j + 1) * 512], ph[:],
                                    b1c[:, j:j + 1], 0.0, op0=ADD, op1=MAX)
        e = sb.tile([128, 1024], F32, tag="e", bufs=2)
        nc.scalar.activation(e[:], u[:], EXP)
        hT = sb.tile([128, 1024], F32R, tag="hT", bufs=2)
        nc.vector.tensor_add(hT[:], r[:], e[:])

        # output projection (+ bias via K=1 ones-matmul)
        if outT is not None:
            po = ps.tile([64, 512], F32, tag="avs")
            nc.tensor.matmul(po[:], bias_lhsT[:], ones512[:],
                             start=True, stop=False)
            for j in range(2):
                nc.tensor.matmul(po[:], w2t[:, j * 64:(j + 1) * 64],
                                 hT[:, j * 512:(j + 1) * 512],
                                 start=False, stop=(j == 1))
            nc.vector.tensor_copy(outT[:, n * 512:(n + 1) * 512], po[:])
        else:
            # stage 2: emit row-major [si, 64] directly to DRAM
            for ss in range(4):
                po2 = ps.tile([128, 64], F32, tag="avs")
                nc.tensor.matmul(po2[:], ones128[:], bias_rhs[:],
                                 start=True, stop=False)
                for j in range(2):
                    nc.tensor.matmul(
                        po2[:],
                        hT[:, j * 512 + ss * 128:j * 512 + (ss + 1) * 128],
                        w2t[:, j * 64:(j + 1) * 64],
                        start=False, stop=(j == 1))
                fin = sb.tile([128, 64], F32, tag="fin", bufs=3)
                nc.vector.tensor_copy(fin[:], po2[:])
                row0 = n * 512 + ss * 128
                nc.sync.dma_start(out_dram[row0:row0 + 128, :], fin[:])


def build_nc():
    nc = bacc.Bacc("TRN2", target_bir_lowering=False, debug=False,
                   num_devices=N_CORES)

    def din(name, shape, dt=F32R):
        return nc.dram_tensor(name, shape, dt, kind="ExternalInput").ap()

    xT_d = din("xT", [64, S])
    wd = {}
    for sfx in ("", "1"):
        wd["wq" + sfx] = din("wq" + sfx, [64, 64])
        wd["wk" + sfx] = din("wk" + sfx, [64, 64])
        wd["wv" + sfx] = din("wv" + sfx, [64, 64])
        wd["w1t" + sfx] = din("w1t" + sfx, [64, HD])
        wd["w2t" + sfx] = din("w2t" + sfx, [HD, 64])
        wd["b1c" + sfx] = din("b1c" + sfx, [128, 2], F32)
        wd["b2" + sfx] = din("b2" + sfx, [1, 64])
    out_d = nc.dram_tensor("out1", [R, 64], F32, kind="ExternalOutput").ap()

    with tile.TileContext(nc) as tc, ExitStack() as ctx:
        consts = ctx.enter_context(tc.tile_pool(name="consts", bufs=1))
        sb = ctx.enter_context(tc.tile_pool(name="sb", bufs=1))
        ps = ctx.enter_context(tc.tile_pool(name="ps", bufs=2, space="PSUM"))
        dram = ctx.enter_context(tc.tile_pool(name="dram", bufs=1,
                                              space="DRAM"))
        pools = (consts, sb, ps)

        ones_f32 = consts.tile([1, 512], F32)
        nc.vector.memset(ones_f32[:], 1.0)
        ones512 = consts.tile([1, 512], F32R)
        nc.vector.tensor_copy(ones512[:], ones_f32[:])
        ones128 = consts.tile([1, 128], F32R)
        nc.vector.tensor_copy(ones128[:], ones_f32[:, 0:128])

        def load_weights(sfx):
            t = {}
            for nm, shp in (("wq", [64, 64]), ("wk", [64, 64]),
                            ("wv", [64, 64]), ("w1t", [64, HD])):
                t[nm] = consts.tile(shp, F32R, name=f"c_{nm}{sfx}")
                nc.sync.dma_start(t[nm][:], wd[nm + sfx])
            t["w2t"] = consts.tile([128, 2, 64], F32R, name=f"c_w2t{sfx}")
            for j in range(2):
                nc.sync.dma_start(t["w2t"][:, j, :],
                                  wd["w2t" + sfx][j * 128:(j + 1) * 128, :])
            t["b1c"] = consts.tile([128, 2], F32, name=f"c_b1c{sfx}")
            nc.sync.dma_start(t["b1c"][:], wd["b1c" + sfx])
            t["b2"] = consts.tile([1, 64], F32R, name=f"c_b2{sfx}")
            nc.sync.dma_start(t["b2"][:], wd["b2" + sfx])
            return (t["wq"], t["wk"], t["wv"], t["w1t"],
                    t["w2t"].rearrange("p j n -> p (j n)"), t["b1c"],
                    t["b2"], t["b2"], ones512, ones128)

        W1 = load_weights("")
        W2 = load_weights("1")

        xT = sb.tile([64, S], F32R, tag="xt")
        nc.sync.dma_start(xT[:], xT_d)

        outT = sb.tile([64, R], F32R, tag="outT")
        _stage(nc, tc, ctx, pools, W1, xT, xT[:, 0:R], outT=outT)

        # pairwise exchange of stage-1 halves
        bounce_in = dram.tile([64, R], F32R)
        bounce_out = dram.tile([2, 64, R], F32R)
        nc.sync.dma_start(bounce_in[:], outT[:])
        nc.gpsimd.collective_compute(
            "AllGather", mybir.AluOpType.bypass,
            replica_groups=[[0, 1], [2, 3], [4, 5], [6, 7]],
            ins=[bounce_in[:].opt()], outs=[bounce_out[:].opt()])
        xT2 = sb.tile([64, S], F32R, tag="xt")
        for m in range(2):
            nc.sync.dma_start(xT2[:, m * R:(m + 1) * R], bounce_out[m])

        _stage(nc, tc, ctx, pools, W2, xT2, outT, out_dram=out_d)

    nc.compile()
    return nc


def prep_inputs(x, q, k, v, q1, k1, v1, W1, b1, W2, b2, W11, b11, W22, b22):
    """Returns per-core in_maps for run_bass_kernel_spmd."""
    f = np.float32

    def cast(a):
        return np.ascontiguousarray(np.asarray(a), dtype=f)

    scale = 1.0 / np.sqrt(np.float32(64))
    common = {
        "wq": cast(q) * f(scale), "wk": cast(k), "wv": cast(v),
        "w1t": cast(W1).T.copy(), "w2t": cast(W2).T.copy(),
        "b1c": cast(b1).reshape(2, 128).T.copy(),
        "b2": (cast(b2) - cast(W2).sum(axis=1)).reshape(1, 64),
        "wq1": cast(q1) * f(scale), "wk1": cast(k1), "wv1": cast(v1),
        "w1t1": cast(W11).T.copy(), "w2t1": cast(W22).T.copy(),
        "b1c1": cast(b11).reshape(2, 128).T.copy(),
        "b21": (cast(b22) - cast(W22).sum(axis=1)).reshape(1, 64),
    }
    in_maps = []
    xc = cast(x)
    for c in range(N_CORES):
        b, h = c // 2, c % 2
        xb = xc[b]                      # [S, 64]
        if h == 1:                      # own half first
            xb = np.concatenate([xb[R:], xb[:R]], axis=0)
        m = dict(common)
        m["xT"] = np.ascontiguousarray(xb.T)
        in_maps.append(m)
    return in_maps


_NC_CACHE = None


def kernel(**inputs) -> np.ndarray:
    global _NC_CACHE
    if _NC_CACHE is None:
        _NC_CACHE = build_nc()
    nc = _NC_CACHE
    in_maps = prep_inputs(**inputs)
    res = bass_utils.run_bass_kernel_spmd(nc, in_maps,
                                          core_ids=list(range(N_CORES)))
    out = np.empty((B, S, 64), dtype=np.float32)
    for c in range(N_CORES):
        b, h = c // 2, c % 2
        out[b, h * R:(h + 1) * R, :] = res.results[c]["out1"]
    return out


# revision 26
# speedup vs baseline: 3.8782x; 3.8782x over previous
"""Trainium2 Bass kernel for nn_AttentionMLP (B=4, S=4096, two attention+MLP stages).

Sharding: 8 cores = 4 batches x 2 sequence-halves. Each core computes its
2048 query rows end-to-end; pairwise AllGathers (chunked, pipelined)
exchange the stage-1 output halves so stage 2 attends over the full
sequence.

Layout strategy (per core, all feature-major / transposed):
  xT [64, S]   -> qT/kT [64, *] projections on PE (fp32r)
  scoresT[j, si] blocks via PE (K=64), exp on ACT into SBUF (fp32r)
  attn@v + rowsum fused: lhsT = [v | ones] [128jb, 65], accumulate in PSUM
  normalize via reciprocal_approx_fast + gpsimd partition_broadcast + DVE mul
  MLP: W1T/W2T matmuls, ELU = max(x+b,0) + exp(min(x+b,0)) - 1 (the -1 is
  folded into the next layer's bias), biases via K=1 ones-matmul into PSUM.

All weights ship in one packed DRAM tensor (single DMA): DMA dispatch costs
~650ns of sequencer time each, so count matters more than bytes here.
"""

import numpy as np
from contextlib import ExitStack

import concourse.bass as bass
import concourse.tile as tile
from concourse import bacc, mybir
from concourse import bass_utils

F32 = mybir.dt.float32
F32R = mybir.dt.float32r
EXP = mybir.ActivationFunctionType.Exp
ADD = mybir.AluOpType.add
MIN = mybir.AluOpType.min
MAX = mybir.AluOpType.max

N_CORES = 8
B, S, D = 4, 4096, 64
R = S // 2            # own query rows per core
HD = 256
NCK = R // 512        # si-chunks per core (4 x 512)
NJB = S // 128        # key blocks (32 x 128)
# exp-group sizes per chunk: one double-buffered [128, 1536] scores tag
# (6 banks) + av (1) + mlp (1).
GROUPS = [3] * 10 + [2]
assert sum(GROUPS) == NJB

# packed-weight column layout (f32 words per partition)
# region A (partitions 0-63, one 448-col block per stage): wq|wk|wv|w1t
WQ0, WK0, WV0, W1T0 = 0, 64, 128, 192
RA = 896
# region B (all 128 partitions): w2t (2 stages x 2 K-blocks x 64) |
# b1c (2 stages x 2 cols) | b2 rows (2 stages x 64, partition 0 only)
W2T0, B1C0, B2R0 = RA, RA + 256, RA + 260
WCOLS = RA + 260 + 128


def _stage(nc, pools, sfx, xT, q_src, outT=None, out_dram=None,
           out_chunk_hook=None):
    """One attention+MLP stage. xT: SBUF [64, S] f32r (full sequence,
    feature-major; may be a pair (tile, ready_fn)). q_src: SBUF [64, R] f32r
    (own rows). Writes outT (SBUF [64, R] f32r, stage 1) or out_dram
    ([R, 64] f32, stage 2). sfx=0/1 selects the weight partition half."""
    sb, ps, wt, ones512, ones128 = pools
    wsl = wt[0:64, sfx * 448:sfx * 448 + 448]
    w2t = wt[:, W2T0 + sfx * 128:W2T0 + sfx * 128 + 128]
    b1c = wt[:, B1C0 + sfx * 2:B1C0 + sfx * 2 + 2].bitcast(F32)
    b2 = wt[0:1, B2R0 + sfx * 64:B2R0 + sfx * 64 + 64]

    # --- projections: emitted lazily inside chunk 0's group loop so the
    # first scores can start as soon as the first kT slice lands, and PSUM
    # slot allocation order never makes scores wait on later projections ---
    qT = sb.tile([64, R], F32R, tag="qT")
    kT = sb.tile([64, S], F32R, tag="kT")
    v_aug = sb.tile([128, NJB, 65], F32R, tag="v_aug")
    onescol = sb.tile([128, NJB], F32, tag="onescol")
    nc.vector.memset(onescol[:], 1.0)
    nc.vector.tensor_copy(v_aug[:, :, 64:65], onescol[:].unsqueeze(2))

    def emit_proj(n):
        sl = slice(n * 512, (n + 1) * 512)
        pk = ps.tile([64, 512], F32, tag="sA", bufs=2)
        nc.tensor.matmul(pk[:], wsl[:, WK0:WK0 + 64], xT[:, sl],
                         start=True, stop=True)
        nc.vector.tensor_copy(kT[:, sl], pk[:])
        if n < R // 512:
            pq = ps.tile([64, 512], F32, tag="sA", bufs=2)
            nc.tensor.matmul(pq[:], wsl[:, WQ0:WQ0 + 64], q_src[:, sl],
                             start=True, stop=True)
            nc.vector.tensor_copy(qT[:, sl], pq[:])
        pv = ps.tile([128, 4, 64], F32, tag="sA", bufs=2)
        for i in range(4):
            jb = n * 4 + i
            nc.tensor.matmul(pv[:, i, :], xT[:, jb * 128:(jb + 1) * 128],
                             wsl[:, WV0:WV0 + 64], start=True, stop=True)
        nc.vector.tensor_copy(v_aug[:, n * 4:(n + 1) * 4, 0:64], pv[:])

    # --- per si-chunk: scores -> exp -> attn@v -> normalize -> MLP ---
    for n in range(NCK):
        qs = qT[:, n * 512:(n + 1) * 512]
        av = None
        jb = 0
        for gi, gsz in enumerate(GROUPS):
            if n == 0 and gi < S // 512:
                emit_proj(gi)
            st = ps.tile([128, gsz * 512], F32, tag="sA", bufs=2)
            for i in range(gsz):
                nc.tensor.matmul(st[:, i * 512:(i + 1) * 512],
                                 kT[:, (jb + i) * 128:(jb + i + 1) * 128],
                                 qs, start=True, stop=True)
            ex = sb.tile([128, gsz * 512], F32R, tag="exp", bufs=3)
            nc.scalar.activation(ex[:], st[:], EXP)
            if av is None:
                av = ps.tile([65, 512], F32, tag="av", bufs=1)
            for i in range(gsz):
                nc.tensor.matmul(av[:], v_aug[:, jb + i, :],
                                 ex[:, i * 512:(i + 1) * 512],
                                 start=(jb + i == 0), stop=(jb + i == NJB - 1))
            jb += gsz

        # normalize: aT = av[0:64] / av[64]. partition_broadcast and the
        # custom recip op only work from partition 0, so move the row sums
        # there first (DVE copies handle cross-partition fine).
        rs = sb.tile([1, 512], F32, tag="rs", bufs=2)
        nc.vector.tensor_copy(rs[:], av[64:65, :])
        rr = sb.tile([1, 512], F32, tag="rr", bufs=2)
        nc.vector.reciprocal_approx_fast(rr[:], rs[:])
        rb = sb.tile([64, 512], F32, tag="rb", bufs=2)
        nc.gpsimd.partition_broadcast(rb[:], rr[:])
        aT = sb.tile([64, 512], F32R, tag="aT", bufs=2)
        nc.vector.tensor_mul(aT[:], av[0:64, :], rb[:])

        # MLP hidden: hT = elu(W1 @ aT + b1) + 1  (the -1 lives in b2_eff)
        u = sb.tile([128, 1024], F32, tag="u", bufs=2)
        r = sb.tile([128, 1024], F32, tag="r", bufs=2)
        for j in range(2):
            ph = ps.tile([128, 512], F32, tag="mlp", bufs=1)
            nc.tensor.matmul(ph[:], wsl[:, W1T0 + j * 128:W1T0 + (j + 1) * 128],
                             aT[:], start=True, stop=True)
            nc.vector.tensor_scalar(u[:, j * 512:(j + 1) * 512], ph[:],
                                    b1c[:, j:j + 1], 0.0, op0=ADD, op1=MIN)
            nc.vector.tensor_scalar(r[:, j * 512:(j + 1) * 512], ph[:],
                                    b1c[:, j:j + 1], 0.0, op0=ADD, op1=MAX)
        e = sb.tile([128, 1024], F32, tag="e", bufs=2)
        nc.scalar.activation(e[:], u[:], EXP)
        hT = sb.tile([128, 1024], F32R, tag="hT", bufs=2)
        nc.vector.tensor_add(hT[:], r[:], e[:])

        # output projection (+ bias via K=1 ones-matmul)
        if outT is not None:
            po = ps.tile([64, 512], F32, tag="mlp", bufs=1)
            nc.tensor.matmul(po[:], b2[:], ones512[:], start=True, stop=False)
            for j in range(2):
                nc.tensor.matmul(po[:], w2t[:, j * 64:(j + 1) * 64],
                                 hT[:, j * 512:(j + 1) * 512],
                                 start=False, stop=(j == 1))
            nc.vector.tensor_copy(outT[:, n * 512:(n + 1) * 512], po[:])
            if out_chunk_hook is not None:
                out_chunk_hook(n)
        else:
            # stage 2: emit row-major [si, 64] directly to DRAM
            for ss in range(4):
                po2 = ps.tile([128, 64], F32, tag="mlp", bufs=1)
                nc.tensor.matmul(po2[:], ones128[:], b2[:],
                                 start=True, stop=False)
                for j in range(2):
                    nc.tensor.matmul(
                        po2[:],
                        hT[:, j * 512 + ss * 128:j * 512 + (ss + 1) * 128],
                        w2t[:, j * 64:(j + 1) * 64],
                        start=False, stop=(j == 1))
                fin = sb.tile([128, 64], F32, tag="fin", bufs=3)
                nc.vector.tensor_copy(fin[:], po2[:])
                row0 = n * 512 + ss * 128
                nc.sync.dma_start(out_dram[row0:row0 + 128, :], fin[:])


def build_nc(n_cores=N_CORES, reps=1):
    nc = bacc.Bacc("TRN2", target_bir_lowering=False, debug=False,
                   num_devices=n_cores)

    xT_d = nc.dram_tensor("xT", [64, S], F32R, kind="ExternalInput").ap()
    w_d = nc.dram_tensor("wpack", [128, WCOLS], F32R,
                         kind="ExternalInput").ap()
    out_d = nc.dram_tensor("out1", [R, 64], F32, kind="ExternalOutput").ap()

    with tile.TileContext(nc) as tc, ExitStack() as ctx:
        consts = ctx.enter_context(tc.tile_pool(name="consts", bufs=1))
        sb = ctx.enter_context(tc.tile_pool(name="sb", bufs=1))
        ps = ctx.enter_context(tc.tile_pool(name="ps", bufs=2, space="PSUM"))
        dram = ctx.enter_context(tc.tile_pool(name="dram", bufs=1,
                                              space="DRAM"))

        wt = consts.tile([128, WCOLS], F32R)
        nc.sync.dma_start(wt[:], w_d)
        ones_f32 = consts.tile([1, 512], F32)
        nc.vector.memset(ones_f32[:], 1.0)
        ones512 = consts.tile([1, 512], F32R)
        nc.vector.tensor_copy(ones512[:], ones_f32[:])
        ones128 = consts.tile([1, 128], F32R)
        nc.vector.tensor_copy(ones128[:], ones_f32[:, 0:128])
        pools = (sb, ps, wt[:], ones512, ones128)

        # spread big loads across the three DMA-dispatch queues
        dma_engines = [nc.sync, nc.scalar, nc.gpsimd]

        for _rep in range(reps):
            xT = sb.tile([64, S], F32R, tag="xt")
            for n in range(S // 512):
                dma_engines[n % 3].dma_start(
                    xT[:, n * 512:(n + 1) * 512],
                    xT_d[:, n * 512:(n + 1) * 512])

            outT = sb.tile([64, R], F32R, tag="outT")
            bounce_ins = [dram.tile([64, 512], F32R,
                                    name=f"bi_{_rep}_{n}", tag=f"bi{n}")
                          for n in range(NCK)]
            bounce_outs = [dram.tile([2, 64, 512], F32R,
                                     name=f"bo_{_rep}_{n}", tag=f"bo{n}")
                           for n in range(NCK)]

            def exchange_chunk(n):
                sl = slice(n * 512, (n + 1) * 512)
                nc.sync.dma_start(bounce_ins[n][:], outT[:, sl])
                if n_cores > 1:
                    nc.gpsimd.collective_compute(
                        "AllGather", mybir.AluOpType.bypass,
                        replica_groups=[[0, 1], [2, 3], [4, 5], [6, 7]],
                        ins=[bounce_ins[n][:].opt()],
                        outs=[bounce_outs[n][:].opt()])
                else:
                    for m in range(2):
                        nc.sync.dma_start(bounce_outs[n][m],
                                          bounce_ins[n][:])

            _stage(nc, pools, 0, xT, xT[:, 0:R], outT=outT,
                   out_chunk_hook=exchange_chunk)

            xT2 = sb.tile([64, S], F32R, tag="xt")
            for m in range(2):
                for n in range(NCK):
                    dma_engines[(m * NCK + n) % 3].dma_start(
                        xT2[:, m * R + n * 512:m * R + (n + 1) * 512],
                        bounce_outs[n][m])

            _stage(nc, pools, 1, xT2, outT, out_dram=out_d)

    nc.compile()
    return nc


def prep_inputs(x, q, k, v, q1, k1, v1, W1, b1, W2, b2, W11, b11, W22, b22):
    """Returns per-core in_maps for run_bass_kernel_spmd."""
    f = np.float32

    def cast(a):
        return np.ascontiguousarray(np.asarray(a), dtype=f)

    scale = f(1.0 / np.sqrt(np.float32(64)))
    wpack = np.zeros((128, WCOLS), dtype=f)
    for sfx, (qq, kk, vv, W1_, b1_, W2_, b2_) in enumerate(
            [(q, k, v, W1, b1, W2, b2), (q1, k1, v1, W11, b11, W22, b22)]):
        c0 = 448 * sfx
        wpack[0:64, c0 + WQ0:c0 + WQ0 + 64] = cast(qq) * scale
        wpack[0:64, c0 + WK0:c0 + WK0 + 64] = cast(kk)
        wpack[0:64, c0 + WV0:c0 + WV0 + 64] = cast(vv)
        wpack[0:64, c0 + W1T0:c0 + W1T0 + HD] = cast(W1_).T
        w2T = cast(W2_).T                                 # [HD, 64]
        for j in range(2):
            wpack[:, W2T0 + sfx * 128 + j * 64:
                  W2T0 + sfx * 128 + (j + 1) * 64] = w2T[j * 128:(j + 1) * 128]
            wpack[:, B1C0 + sfx * 2 + j] = cast(b1_)[j * 128:(j + 1) * 128]
        wpack[0, B2R0 + sfx * 64:B2R0 + (sfx + 1) * 64] = \
            cast(b2_) - cast(W2_).sum(axis=1)

    in_maps = []
    xc = cast(x)
    for c in range(N_CORES):
        b, h = c // 2, c % 2
        xb = xc[b]                      # [S, 64]
        if h == 1:                      # own half first
            xb = np.concatenate([xb[R:], xb[:R]], axis=0)
        in_maps.append({"xT": np.ascontiguousarray(xb.T), "wpack": wpack})
    return in_maps


_NC_CACHE = None


def kernel(**inputs) -> np.ndarray:
    global _NC_CACHE
    if _NC_CACHE is None:
        _NC_CACHE = build_nc()
    nc = _NC_CACHE
    in_maps = prep_inputs(**inputs)
    res = bass_utils.run_bass_kernel_spmd(nc, in_maps,
                                          core_ids=list(range(N_CORES)))
    out = np.empty((B, S, 64), dtype=np.float32)
    for c in range(N_CORES):
        b, h = c // 2, c % 2
        out[b, h * R:(h + 1) * R, :] = res.results[c]["out1"]
    return out
